# revision 33
# baseline (speedup 1.0000x reference)
"""Trainium2 Bass kernel for nn_ContrastiveLoss (SimCLR-style, N=8192, D=128).

v5: host-normalized d-major input + DVE colsum/rowsum + quadrant half-pair.

Host normalizes z and ships znT = zn.T per core as bf16 [128(d), 5120 cols]
(10KB contiguous per partition -> ~256 total DMA descriptors over the two
HWDGE rings instead of v4's 640).  On device each core computes
e = exp(10 cos - 10) for its 8 stationary sub-blocks (own 1024 rows) times
5120 local columns:
  cols [0:1024]    own block (diag)        rowsum only
  cols [1024:4096] blocks c+1..c+3         rowsum + colsum
  cols [4096:5120] the {c,c+4} pair block, quadrant-split so each endpoint
                   computes half the pairs: sub-row k does cols
                   [4096+(k%2)*512, +512)  rowsum + colsum
The {c,c+4} quadrant split keeps one program for all cores: the host places
own rows interleaved (even subs = own[0:512], odd subs = own[512:1024]) and
picks the partner half-columns per core, so sets
  evens x H0  +  odds x H1   (this core)
  evens x H0  +  odds x H1   (partner core, complementary halves)
partition the 1024x1024 pair block exactly.  W = 36864 exp-cols/core vs
v4's 40960.

ACT is the critical engine: 24 exp instrs (8x2048 + 8x2048 + 8x512),
no accum reads (rowsums via DVE 4x-mode reduces over the persistent bf16
ej slabs, colsums via DVE bf16 adds into csacc + one final ones-matmul
pass).  Outputs are descriptor-light: cs [1,4096] (1 desc) and a
PE-transposed rs [8,128] (8 descs).
"""

import sys

sys.path.insert(0, "/opt/trn_rl_repo")

from contextlib import ExitStack

import numpy as np
import ml_dtypes

import concourse.bass as bass
import concourse.bacc as bacc
import concourse.tile as tile
from concourse import mybir
from concourse import bass_utils
from concourse.masks import make_identity

B = 4096
D = 128
N = 2 * B            # 8192 rows of z
NCORES = 8
ROWS = N // NCORES   # 1024 rows per core
NBLK = ROWS // 128   # 8 stationary sub-blocks per core
COLS = 5120          # local columns kept per core
HP = 4096            # half-pair region start
EJW = 4608           # per-sub-row ej width: 2048 + 2048 + 512
INV_T = 10.0         # 1/temperature

F32 = mybir.dt.float32
BF16 = mybir.dt.bfloat16
AX = mybir.AxisListType
AF = mybir.ActivationFunctionType
OP = mybir.AluOpType


def _build() -> bass.Bass:
    nc = bacc.Bacc(None)
    z_in = nc.declare_dram_parameter("z", [128, COLS], BF16, isOutput=False)
    out_cs = nc.declare_dram_parameter("cs", [1, 4608], F32, isOutput=True)
    out_rs = nc.declare_dram_parameter("rs", [NBLK, 128], F32, isOutput=True)

    with tile.TileContext(nc) as tc:
        with ExitStack() as ctx:
            persist = ctx.enter_context(tc.tile_pool(name="persist", bufs=1))
            psum = ctx.enter_context(tc.tile_pool(name="psum", bufs=2, space="PSUM"))

            znT = persist.tile([128, COLS], BF16)
            # input DMA partition-split across the two HWDGE rings
            nc.sync.dma_start(out=znT[0:64, 0:2048], in_=z_in[0:64, 0:2048])
            nc.scalar.dma_start(out=znT[64:128, 0:2048], in_=z_in[64:128, 0:2048])
            nc.sync.dma_start(out=znT[0:64, 2048:COLS], in_=z_in[0:64, 2048:COLS])
            nc.scalar.dma_start(out=znT[64:128, 2048:COLS], in_=z_in[64:128, 2048:COLS])

            b_neg10 = persist.tile([128, 1], F32)
            nc.vector.memset(b_neg10, -INV_T)
            ones_col = persist.tile([128, 1], BF16)
            nc.vector.memset(ones_col, 1.0)
            ident = persist.tile([128, 128], F32)
            make_identity(nc, ident)
            # prime the exp table set while the input DMA streams
            prime = persist.tile([128, 1], F32)
            nc.scalar.activation(prime, b_neg10, AF.Exp, bias=b_neg10)

            ej = [
                persist.tile([128, EJW], BF16, name=f"ej{b}") for b in range(NBLK)
            ]
            csacc = persist.tile([128, 3072], BF16)
            csd = persist.tile([128, 512], BF16)
            acc = persist.tile([128, NBLK, 2], F32)
            rs3 = persist.tile([128, NBLK, 4], BF16)
            rs3f = persist.tile([128, NBLK], F32)
            rs_a = persist.tile([128, NBLK], F32)
            rs = persist.tile([128, NBLK], F32)
            cs_sb = persist.tile([1, 4608], F32)
            rs_sb = persist.tile([NBLK, 128], F32)

            def stat(b):
                return znT[:, b * 128:(b + 1) * 128]

            # ---- phase 0: cols [0:2048] (diag + first colsum chunk) ----
            # sub-rows 4-7 skip diag cols [0:512] (those pairs are computed
            # by sub-rows 0-3, recovered for S via the csd colsum)
            for b in range(NBLK):
                st = 512 if b >= 4 else 0
                w = 2048 - st
                pt = psum.tile([128, 2048], F32, tag="pp", name="pt")
                for s in range(w // 512):
                    nc.tensor.matmul(
                        pt[:, s * 512:(s + 1) * 512],
                        stat(b),
                        znT[:, st + s * 512:st + (s + 1) * 512],
                        start=True,
                        stop=True,
                    )
                nc.scalar.activation(
                    ej[b][:, st:2048], pt[:, 0:w], AF.Exp, scale=INV_T,
                    bias=b_neg10, accum_out=acc[:, b, 0:1],
                )
                if b == 0:
                    nc.vector.tensor_copy(csacc[:, 0:1024], ej[b][:, 1024:2048])
                    nc.vector.tensor_copy(csd, ej[b][:, 512:1024])
                else:
                    nc.vector.tensor_add(
                        csacc[:, 0:1024], csacc[:, 0:1024], ej[b][:, 1024:2048]
                    )
                    if b < 4:
                        nc.vector.tensor_add(csd, csd, ej[b][:, 512:1024])

            # ---- phase 1: cols [2048:4096] (colsum chunks 2,3) ----------
            for b in range(NBLK):
                pt = psum.tile([128, 2048], F32, tag="pp", name="pt")
                for s in range(4):
                    nc.tensor.matmul(
                        pt[:, s * 512:(s + 1) * 512],
                        stat(b),
                        znT[:, 2048 + s * 512:2048 + (s + 1) * 512],
                        start=True,
                        stop=True,
                    )
                nc.scalar.activation(
                    ej[b][:, 2048:4096], pt, AF.Exp, scale=INV_T, bias=b_neg10,
                    accum_out=acc[:, b, 1:2],
                )
                if b == 0:
                    nc.vector.tensor_copy(csacc[:, 1024:3072], ej[b][:, 2048:4096])
                else:
                    nc.vector.tensor_add(
                        csacc[:, 1024:3072], csacc[:, 1024:3072], ej[b][:, 2048:4096]
                    )


            # ---- phase 2: half-pair cols + final colsum matmuls ---------
            # 4 sub-rows share one psum tile (4 banks) so the exp stream
            # never waits on a cs_emit slot-hold; cs work rides the slot
            # freed by the previous group's exps
            def p2_group(g):
                p2t = psum.tile([128, 2048], F32, tag="pp", name="p2t")
                for q in range(4):
                    b = g * 4 + q
                    h = HP + (b % 2) * 512
                    nc.tensor.matmul(
                        p2t[:, q * 512:(q + 1) * 512],
                        stat(b), znT[:, h:h + 512], start=True, stop=True,
                    )
                for q in range(4):
                    b = g * 4 + q
                    nc.scalar.activation(
                        ej[b][:, 4096:4608], p2t[:, q * 512:(q + 1) * 512],
                        AF.Exp, scale=INV_T, bias=b_neg10,
                    )
                    with nc.allow_low_precision("bf16 partials of exp<=1 sums"):
                        nc.vector.reduce_sum(
                            rs3[:, b, :],
                            ej[b][:, 4096:4608].rearrange("p (g x) -> p g x", x=128),
                            axis=AX.X,
                        )

            p2_group(0)
            # colsum chunks 0-3 (csacc[0:2048], ready after phase 1); copies
            # run mid-phase-2 so the DVE FIFO doesn't trap them at the end
            cpt1 = psum.tile([128, 2048], F32, tag="pp", name="cpt1")
            for j in range(4):
                nc.tensor.matmul(
                    cpt1[0:1, j * 512:(j + 1) * 512],
                    ones_col, csacc[:, j * 512:(j + 1) * 512],
                    start=True, stop=True,
                )
            nc.vector.tensor_copy(cs_sb[:, 0:1024], cpt1[0:1, 0:1024])
            nc.vector.tensor_copy(cs_sb[:, 1024:2048], cpt1[0:1, 1024:2048])
            p2_group(1)

            # ---- tail: cs chunks 4,5 + csd + hp colsums (PE-accumulated) -
            cpt2 = psum.tile([128, 2048], F32, tag="pp", name="cpt2")
            for j in (4, 5):
                nc.tensor.matmul(
                    cpt2[0:1, (j - 4) * 512:(j - 3) * 512],
                    ones_col, csacc[:, j * 512:(j + 1) * 512],
                    start=True, stop=True,
                )
            for par in range(2):         # hp colsums straight off the ej slabs
                bs = [par, par + 2, par + 4, par + 6]
                for n, b in enumerate(bs):
                    nc.tensor.matmul(
                        cpt2[0:1, 1024 + par * 512:1536 + par * 512],
                        ones_col, ej[b][:, 4096:4608],
                        start=(n == 0), stop=(n == 3),
                    )
            nc.vector.reduce_sum(rs_a, acc, axis=AX.X)
            with nc.allow_low_precision("combine"):
                nc.vector.reduce_sum(rs3f, rs3, axis=AX.X)
            nc.vector.tensor_add(rs, rs_a, rs3f)
            rstt = psum.tile([128, 2048], F32, tag="pp", name="rstt")
            rst = rstt[0:NBLK, 0:128]
            nc.tensor.transpose(rst, rs, ident)
            nc.tensor.matmul(                  # csd colsum rides rstt bank 1
                rstt[0:1, 512:1024], ones_col, csd, start=True, stop=True,
            )
            nc.vector.tensor_copy(rs_sb, rst)
            nc.vector.tensor_copy(cs_sb[:, 2048:3072], cpt2[0:1, 0:1024])
            nc.scalar.copy(cs_sb[:, 3072:4096], cpt2[0:1, 1024:2048])
            nc.vector.tensor_copy(cs_sb[:, 4096:4608], rstt[0:1, 512:1024])
            nc.scalar.dma_start(out=out_rs[:, :], in_=rs_sb)
            nc.sync.dma_start(out=out_cs[:, :], in_=cs_sb)

    nc.compile()
    return nc


_NC = None


def _get_nc() -> bass.Bass:
    global _NC
    if _NC is None:
        _NC = _build()
    return _NC


def _base_k(k: int) -> int:
    return (k // 2) * 128 + (512 if k % 2 else 0)


def make_in_maps(zn: np.ndarray) -> list[dict]:
    """zn: [8192, 128] float32, already L2-normalized."""
    zn16 = zn.astype(ml_dtypes.bfloat16)
    maps = []
    for c in range(NCORES):
        own = c * ROWS
        cols = []
        for k in range(NBLK):
            cols.append(own + _base_k(k) + np.arange(128))
        cols.append((own + 1024 + np.arange(3072)) % N)
        p = ((c + 4) % 8) * ROWS
        if c < 4:
            cols.append(p + np.arange(512))
            cols.append(p + 512 + np.arange(512))
        else:
            cols.append(p + 512 + np.arange(512))
            cols.append(p + np.arange(512))
        idx = np.concatenate(cols)
        znT = np.ascontiguousarray(zn16[idx].T)   # [128, 5120]
        maps.append({"z": znT})
    return maps


def kernel(emb0: np.ndarray, emb1: np.ndarray) -> np.ndarray:
    z = np.concatenate(
        [np.asarray(emb0, np.float32), np.asarray(emb1, np.float32)], axis=0
    )
    nrm = np.maximum(np.sqrt((z * z).sum(axis=1, keepdims=True)), 1e-8)
    zn = z / nrm
    res = bass_utils.run_bass_kernel_spmd(
        _get_nc(), make_in_maps(zn), core_ids=list(range(NCORES))
    )
    # assemble full row sums of exp(10 cos - 10) from per-core partials
    S = np.zeros(N, dtype=np.float64)
    for c, r in enumerate(res.results):
        rs = r["rs"].astype(np.float64)           # [8, 128]
        cs = r["cs"].astype(np.float64).reshape(-1)  # [4608]
        for k in range(NBLK):
            S[c * ROWS + _base_k(k): c * ROWS + _base_k(k) + 128] += rs[k]
        idx = (c * ROWS + 1024 + np.arange(3072)) % N
        np.add.at(S, idx, cs[0:3072])
        p = ((c + 4) % 8) * ROWS
        if c < 4:
            S[p:p + 512] += cs[3072:3584]
            S[p + 512:p + 1024] += cs[3584:4096]
        else:
            S[p + 512:p + 1024] += cs[3072:3584]
            S[p:p + 512] += cs[3584:4096]
        # csd: diag pairs skipped by sub-rows 4-7 (their cols [0:512]),
        # recovered as colsums over sub-rows 0-3 at col positions [512:1024]
        for k in range(4, NBLK):
            S[c * ROWS + _base_k(k): c * ROWS + _base_k(k) + 128] += (
                cs[4096 + (k - 4) * 128: 4224 + (k - 4) * 128]
            )
    S -= 1.0  # remove the diagonal term exp(10*1 - 10) = 1
    g_pos = np.einsum("ij,ij->i", zn, np.roll(zn, -B, axis=0)).sum(dtype=np.float64)
    total = float(np.sum(np.log(S))) + INV_T * N - INV_T * g_pos
    return np.asarray(np.float32(total / N))


# revision 36
# speedup vs baseline: 1.2240x; 1.2240x over previous
"""Trainium2 Bass kernel for nn_ContrastiveLoss (SimCLR-style, N=8192, D=128).

v5: host-normalized d-major input + DVE colsum/rowsum + quadrant half-pair.

Host normalizes z and ships znT = zn.T per core as bf16 [128(d), 5120 cols]
(10KB contiguous per partition -> ~256 total DMA descriptors over the two
HWDGE rings instead of v4's 640).  On device each core computes
e = exp(10 cos - 10) for its 8 stationary sub-blocks (own 1024 rows) times
5120 local columns:
  cols [0:1024]    own block (diag)        rowsum only
  cols [1024:4096] blocks c+1..c+3         rowsum + colsum
  cols [4096:5120] the {c,c+4} pair block, quadrant-split so each endpoint
                   computes half the pairs: sub-row k does cols
                   [4096+(k%2)*512, +512)  rowsum + colsum
The {c,c+4} quadrant split keeps one program for all cores: the host places
own rows interleaved (even subs = own[0:512], odd subs = own[512:1024]) and
picks the partner half-columns per core, so sets
  evens x H0  +  odds x H1   (this core)
  evens x H0  +  odds x H1   (partner core, complementary halves)
partition the 1024x1024 pair block exactly.  W = 36864 exp-cols/core vs
v4's 40960.

ACT is the critical engine: 24 exp instrs (8x2048 + 8x2048 + 8x512),
no accum reads (rowsums via DVE 4x-mode reduces over the persistent bf16
ej slabs, colsums via DVE bf16 adds into csacc + one final ones-matmul
pass).  Outputs are descriptor-light: cs [1,4096] (1 desc) and a
PE-transposed rs [8,128] (8 descs).
"""

import sys

sys.path.insert(0, "/opt/trn_rl_repo")

from contextlib import ExitStack

import numpy as np
import ml_dtypes

import concourse.bass as bass
import concourse.bacc as bacc
import concourse.tile as tile
from concourse import mybir
from concourse import bass_utils
from concourse.masks import make_identity

B = 4096
D = 128
N = 2 * B            # 8192 rows of z
NCORES = 8
ROWS = N // NCORES   # 1024 rows per core
NBLK = ROWS // 128   # 8 stationary sub-blocks per core
COLS = 5120          # local columns kept per core
HP = 4096            # half-pair region start
EJW = 4608           # per-sub-row ej width: 2048 + 2048 + 512
INV_T = 10.0         # 1/temperature

F32 = mybir.dt.float32
BF16 = mybir.dt.bfloat16
AX = mybir.AxisListType
AF = mybir.ActivationFunctionType
OP = mybir.AluOpType


def _build() -> bass.Bass:
    nc = bacc.Bacc(None)
    z_in = nc.declare_dram_parameter("z", [128, COLS], BF16, isOutput=False)
    out_cs = nc.declare_dram_parameter("cs", [1, 4608], F32, isOutput=True)
    out_rs = nc.declare_dram_parameter("rs", [NBLK, 128], F32, isOutput=True)

    with tile.TileContext(nc) as tc:
        with ExitStack() as ctx:
            persist = ctx.enter_context(tc.tile_pool(name="persist", bufs=1))
            psum = ctx.enter_context(tc.tile_pool(name="psum", bufs=2, space="PSUM"))

            znT = persist.tile([128, COLS], BF16)
            # input DMA partition-split across the two HWDGE rings
            nc.sync.dma_start(out=znT[0:64, 0:2048], in_=z_in[0:64, 0:2048])
            nc.scalar.dma_start(out=znT[64:128, 0:2048], in_=z_in[64:128, 0:2048])
            nc.sync.dma_start(out=znT[0:64, 2048:COLS], in_=z_in[0:64, 2048:COLS])
            nc.scalar.dma_start(out=znT[64:128, 2048:COLS], in_=z_in[64:128, 2048:COLS])

            b_neg10 = persist.tile([128, 1], F32)
            nc.vector.memset(b_neg10, -INV_T)
            ones_col = persist.tile([128, 1], BF16)
            nc.vector.memset(ones_col, 1.0)
            ident = persist.tile([128, 128], F32)
            make_identity(nc, ident)
            # prime the exp table set while the input DMA streams
            prime = persist.tile([128, 1], F32)
            nc.scalar.activation(prime, b_neg10, AF.Exp, bias=b_neg10)

            ej = [
                persist.tile([128, EJW], BF16, name=f"ej{b}") for b in range(NBLK)
            ]
            csacc = persist.tile([128, 3072], BF16)
            csd = persist.tile([128, 512], BF16)
            acc = persist.tile([128, NBLK, 2], F32)
            rs3 = persist.tile([128, NBLK, 4], BF16)
            rs3f = persist.tile([128, NBLK], F32)
            rs_a = persist.tile([128, NBLK], F32)
            rs = persist.tile([128, NBLK], F32)
            cs_sb = persist.tile([1, 4608], F32)
            rs_sb = persist.tile([NBLK, 128], F32)

            def stat(b):
                return znT[:, b * 128:(b + 1) * 128]

            # ---- phase 0: cols [0:2048] (diag + first colsum chunk) ----
            # sub-rows 4-7 skip diag cols [0:512] (those pairs are computed
            # by sub-rows 0-3, recovered for S via the csd colsum)
            for b in range(NBLK):
                st = 512 if b >= 4 else 0
                w = 2048 - st
                pt = psum.tile([128, 2048], F32, tag="pp", name="pt")
                for s in range(w // 512):
                    nc.tensor.matmul(
                        pt[:, s * 512:(s + 1) * 512],
                        stat(b),
                        znT[:, st + s * 512:st + (s + 1) * 512],
                        start=True,
                        stop=True,
                    )
                nc.scalar.activation(
                    ej[b][:, st:2048], pt[:, 0:w], AF.Exp, scale=INV_T,
                    bias=b_neg10, accum_out=acc[:, b, 0:1],
                )
                if b == 0:
                    nc.vector.tensor_copy(csacc[:, 0:1024], ej[b][:, 1024:2048])
                    nc.vector.tensor_copy(csd, ej[b][:, 512:1024])
                else:
                    nc.vector.tensor_add(
                        csacc[:, 0:1024], csacc[:, 0:1024], ej[b][:, 1024:2048]
                    )
                    if b < 4:
                        nc.vector.tensor_add(csd, csd, ej[b][:, 512:1024])

            # ---- phase 1: cols [2048:4096] (colsum chunks 2,3) ----------
            for b in range(NBLK):
                pt = psum.tile([128, 2048], F32, tag="pp", name="pt")
                for s in range(4):
                    nc.tensor.matmul(
                        pt[:, s * 512:(s + 1) * 512],
                        stat(b),
                        znT[:, 2048 + s * 512:2048 + (s + 1) * 512],
                        start=True,
                        stop=True,
                    )
                nc.scalar.activation(
                    ej[b][:, 2048:4096], pt, AF.Exp, scale=INV_T, bias=b_neg10,
                    accum_out=acc[:, b, 1:2],
                )
                if b == 0:
                    nc.vector.tensor_copy(csacc[:, 1024:3072], ej[b][:, 2048:4096])
                else:
                    nc.vector.tensor_add(
                        csacc[:, 1024:3072], csacc[:, 1024:3072], ej[b][:, 2048:4096]
                    )


            # ---- phase 2: half-pair cols + final colsum matmuls ---------
            # 4 sub-rows share one psum tile (4 banks) so the exp stream
            # never waits on a cs_emit slot-hold; cs work rides the slot
            # freed by the previous group's exps
            def p2_group(g):
                p2t = psum.tile([128, 2048], F32, tag="pp", name="p2t")
                for q in range(4):
                    b = g * 4 + q
                    h = HP + (b % 2) * 512
                    nc.tensor.matmul(
                        p2t[:, q * 512:(q + 1) * 512],
                        stat(b), znT[:, h:h + 512], start=True, stop=True,
                    )
                for q in range(4):
                    b = g * 4 + q
                    nc.scalar.activation(
                        ej[b][:, 4096:4608], p2t[:, q * 512:(q + 1) * 512],
                        AF.Exp, scale=INV_T, bias=b_neg10,
                    )
                    with nc.allow_low_precision("bf16 partials of exp<=1 sums"):
                        nc.vector.reduce_sum(
                            rs3[:, b, :],
                            ej[b][:, 4096:4608].rearrange("p (g x) -> p g x", x=128),
                            axis=AX.X,
                        )

            nc.vector.reduce_sum(rs_a, acc, axis=AX.X)  # acc complete here
            p2_group(0)
            # colsum chunks 0-3 (csacc[0:2048], ready after phase 1); copies
            # run mid-phase-2 so the DVE FIFO doesn't trap them at the end
            cpt1 = psum.tile([128, 2048], F32, tag="pp", name="cpt1")
            for j in range(4):
                nc.tensor.matmul(
                    cpt1[0:1, j * 512:(j + 1) * 512],
                    ones_col, csacc[:, j * 512:(j + 1) * 512],
                    start=True, stop=True,
                )
            nc.vector.tensor_copy(cs_sb[:, 0:1024], cpt1[0:1, 0:1024])
            nc.vector.tensor_copy(cs_sb[:, 1024:2048], cpt1[0:1, 1024:2048])
            p2_group(1)

            # ---- tail: cs chunks 4,5 + csd + hp colsums (PE-accumulated) -
            cpt2 = psum.tile([128, 2048], F32, tag="pp", name="cpt2")
            for j in (4, 5):
                nc.tensor.matmul(
                    cpt2[0:1, (j - 4) * 512:(j - 3) * 512],
                    ones_col, csacc[:, j * 512:(j + 1) * 512],
                    start=True, stop=True,
                )
            for par in range(2):         # hp colsums straight off the ej slabs
                bs = [par, par + 2, par + 4, par + 6]
                for n, b in enumerate(bs):
                    nc.tensor.matmul(
                        cpt2[0:1, 1024 + par * 512:1536 + par * 512],
                        ones_col, ej[b][:, 4096:4608],
                        start=(n == 0), stop=(n == 3),
                    )
            with nc.allow_low_precision("combine"):
                nc.vector.reduce_sum(rs3f, rs3, axis=AX.X)
            nc.vector.tensor_add(rs, rs_a, rs3f)
            rstt = psum.tile([128, 2048], F32, tag="pp", name="rstt")
            rst = rstt[0:NBLK, 0:128]
            nc.tensor.transpose(rst, rs, ident)
            nc.tensor.matmul(                  # csd colsum rides rstt bank 1
                rstt[0:1, 512:1024], ones_col, csd, start=True, stop=True,
            )
            nc.vector.tensor_copy(rs_sb, rst)
            nc.vector.tensor_copy(cs_sb[:, 2048:3072], cpt2[0:1, 0:1024])
            nc.scalar.copy(cs_sb[:, 3072:3584], cpt2[0:1, 1024:1536])
            nc.scalar.copy(cs_sb[:, 3584:4096], cpt2[0:1, 1536:2048])
            nc.vector.tensor_copy(cs_sb[:, 4096:4608], rstt[0:1, 512:1024])
            nc.scalar.dma_start(out=out_rs[:, :], in_=rs_sb)
            nc.sync.dma_start(out=out_cs[:, :], in_=cs_sb)

    nc.compile()
    return nc


_NC = None


def _get_nc() -> bass.Bass:
    global _NC
    if _NC is None:
        _NC = _build()
    return _NC


def _base_k(k: int) -> int:
    return (k // 2) * 128 + (512 if k % 2 else 0)


def make_in_maps(zn: np.ndarray) -> list[dict]:
    """zn: [8192, 128] float32, already L2-normalized."""
    zn16 = zn.astype(ml_dtypes.bfloat16)
    maps = []
    for c in range(NCORES):
        own = c * ROWS
        cols = []
        for k in range(NBLK):
            cols.append(own + _base_k(k) + np.arange(128))
        cols.append((own + 1024 + np.arange(3072)) % N)
        p = ((c + 4) % 8) * ROWS
        if c < 4:
            cols.append(p + np.arange(512))
            cols.append(p + 512 + np.arange(512))
        else:
            cols.append(p + 512 + np.arange(512))
            cols.append(p + np.arange(512))
        idx = np.concatenate(cols)
        znT = np.ascontiguousarray(zn16[idx].T)   # [128, 5120]
        maps.append({"z": znT})
    return maps


def kernel(emb0: np.ndarray, emb1: np.ndarray) -> np.ndarray:
    z = np.concatenate(
        [np.asarray(emb0, np.float32), np.asarray(emb1, np.float32)], axis=0
    )
    nrm = np.maximum(np.sqrt((z * z).sum(axis=1, keepdims=True)), 1e-8)
    zn = z / nrm
    res = bass_utils.run_bass_kernel_spmd(
        _get_nc(), make_in_maps(zn), core_ids=list(range(NCORES))
    )
    # assemble full row sums of exp(10 cos - 10) from per-core partials
    S = np.zeros(N, dtype=np.float64)
    for c, r in enumerate(res.results):
        rs = r["rs"].astype(np.float64)           # [8, 128]
        cs = r["cs"].astype(np.float64).reshape(-1)  # [4608]
        for k in range(NBLK):
            S[c * ROWS + _base_k(k): c * ROWS + _base_k(k) + 128] += rs[k]
        idx = (c * ROWS + 1024 + np.arange(3072)) % N
        np.add.at(S, idx, cs[0:3072])
        p = ((c + 4) % 8) * ROWS
        if c < 4:
            S[p:p + 512] += cs[3072:3584]
            S[p + 512:p + 1024] += cs[3584:4096]
        else:
            S[p + 512:p + 1024] += cs[3072:3584]
            S[p:p + 512] += cs[3584:4096]
        # csd: diag pairs skipped by sub-rows 4-7 (their cols [0:512]),
        # recovered as colsums over sub-rows 0-3 at col positions [512:1024]
        for k in range(4, NBLK):
            S[c * ROWS + _base_k(k): c * ROWS + _base_k(k) + 128] += (
                cs[4096 + (k - 4) * 128: 4224 + (k - 4) * 128]
            )
    S -= 1.0  # remove the diagonal term exp(10*1 - 10) = 1
    g_pos = np.einsum("ij,ij->i", zn, np.roll(zn, -B, axis=0)).sum(dtype=np.float64)
    total = float(np.sum(np.log(S))) + INV_T * N - INV_T * g_pos
    return np.asarray(np.float32(total / N))
